# revision 34
# baseline (speedup 1.0000x reference)
"""Heterogeneous-graph SAGEConv (3 node types, 9 bipartite edge sets) on 8 TRN2 cores.

Strategy: shard destination nodes across the 8 cores (graph parallel),
with a per-dst-type balanced permutation that equalizes every
(tile, src-type, src-quarter) in-degree across cores, so the SPMD
slot structure carries almost no equalization padding. The host
partitions each edge list by (dst-core, tile-group-of-7x128,
src-quarter) into tightly packed slot runs (no per-tile padding) and
ships wrapped int16 gather indices plus per-slot one-hot ids. Source
features are replicated as fp16 rows padded to 256B so dma_gather
(256B min element) returns matmul-ready fp16 with no on-device cast.

Device pipeline per (dst-type j, tile-group g):
  one big dma_gather per (src-pair i, quarter q) covering the whole
      group (merged => ~1us SWDGE fixed cost amortized over ~2.3k
      descriptors; ragged num_idxs so pad slots cost no descriptors)
      -> xg_i [128 edge-slots x 128] fp16 (cols 0:64 are features)
  per tile t, per i, per 128-slot chunk intersecting the tile:
      DVE is_equal vs iota bank -> one-hot [128 x 128] fp16. Slot ids
      are dloc + 128*(segment ordinal within chunk), so chunks shared
      by several tiles disambiguate via one of 4 iota banks and every
      matmul contracts the full 128 partitions.
      PE matmul ps_i[64f x 128d] += xg_chunk^T @ onehot
  recip(deg) broadcast via PE outer product from a host-precomputed
      [1, NT*384] bf16 image; ACT copy -> rb
  mean_i = DVE mult(ps_i, rb) -> bf16
  stage-2 PE: ps2[64 x 128] = Cp_j @ xt16_tile + sum_i A_ij @ mean_i
      (A_ij = Wl_ij^T linW_j,i^T ; Cp_j rows 0..63 = sum_i Wr_ij^T linW_j,i^T,
       row 64 = folded bias; folded on device from the raw weights)
  ACT copy -> osb group buffer, one DMA per (j, group) to DRAM.

Host inverts the balanced permutation into the final [3, N, 64].
"""

import sys

import numpy as np

for p in ("/opt/trn_rl_repo", "/root/.axon_site/_ro/trn_rl_repo"):
    if p not in sys.path:
        sys.path.append(p)

import concourse.bacc as bacc_mod  # noqa: E402
import concourse.bass as bass  # noqa: E402
import concourse.mybir as mybir  # noqa: E402
from concourse.bass_utils import run_bass_kernel_spmd  # noqa: E402
from concourse.tile import TileContext  # noqa: E402

F32 = mybir.dt.float32
BF16 = mybir.dt.bfloat16
FP16 = mybir.dt.float16
I16 = mybir.dt.int16
NP_BF16 = mybir.dt.np(BF16)
NP_FP16 = np.float16

PAD_ID = 1600.0  # slot id for pads: no iota-bank match -> zero one-hot row


def default_cfg():
    return dict(C=3, N=100000, D=64, NCORES=8, NQ=4, G=7)


def _derive(cfg):
    c = dict(cfg)
    c["NSH"] = c["N"] // c["NCORES"]          # dst nodes per core
    c["NT"] = (c["NSH"] + 127) // 128          # dst tiles per core
    c["NTP"] = c["NT"] * 128                   # padded dst per core
    c["QS"] = (c["N"] + c["NQ"] - 1) // c["NQ"]  # src rows per quarter (<=32767)
    assert c["QS"] <= 32767
    assert c["NT"] % c["G"] == 0
    c["NG"] = c["NT"] // c["G"]                # tile groups per core
    c["DEGC"] = 3 * c["NT"]                    # recip image cols: 128*3NT
    return c


# ---------------------------------------------------------------- host prep
def _prep_pair(cfg, e):
    """Per (i,j) pair: core-equalized slot structure + per-core images.

    Slot order (per core): group-major, then quarter, then tile-in-group,
    each (g,q,t) segment padded to a multiple of 128 slots (chunks).
    """
    NCORES, NSH, NQ, QS, G, NG = (
        cfg["NCORES"], cfg["NSH"], cfg["NQ"], cfg["QS"], cfg["G"], cfg["NG"])
    src = np.asarray(e[0], dtype=np.int64)
    dst = np.asarray(e[1], dtype=np.int64)
    core = dst // NSH
    dlc = dst % NSH
    tile = dlc // 128
    dloc = (dlc % 128).astype(np.float32)
    g = tile // G
    tin = tile % G
    q = src // QS
    qi = (src % QS).astype(np.int16)

    key = (((core * NG + g) * NQ + q) * G + tin)
    order = np.argsort(key, kind="stable")
    qi_s = qi[order]
    dloc_s = dloc[order]

    nseg = NCORES * NG * NQ * G
    seg = np.bincount(key[order], minlength=nseg).reshape(NCORES, NG * NQ * G)
    # 32-aligned per-(g,q,t) slot allocations, packed tight within each
    # (g,q) quarter-run; runs pad to 128 (gather rectangularity) but
    # num_idxs stops at run_len so tail pads cost no descriptors.
    alloc = seg.max(axis=0).reshape(NG, NQ, G)
    assert (alloc > 0).all()
    run_len = alloc.sum(axis=2)                       # [NG, NQ]
    run_alloc = (run_len + 127) // 128 * 128          # [NG, NQ]
    run_base = np.concatenate(
        [[0], np.cumsum(run_alloc.ravel())])[:-1].reshape(NG, NQ)
    seg_off = run_base[:, :, None] + np.concatenate(
        [np.zeros((NG, NQ, 1), dtype=np.int64),
         np.cumsum(alloc, axis=2)[:, :, :-1]], axis=2)
    S = int(run_alloc.sum())
    M = S // 128

    core_tot = seg.sum(axis=1)
    core_off = np.concatenate([[0], np.cumsum(core_tot)])
    starts_global = seg_off.ravel()
    gidx_imgs, ids_imgs = [], []
    for cidx in range(NCORES):
        a, b = core_off[cidx], core_off[cidx + 1]
        cnts = seg[cidx]
        seg_starts = np.concatenate([[0], np.cumsum(cnts)])[:-1]
        rank = np.arange(b - a) - np.repeat(seg_starts, cnts)
        st_e = np.repeat(starts_global, cnts)
        pos = st_e + rank
        # iota-bank ordinal: rank of the slot's segment among segments
        # starting inside the slot's chunk (continuing segments rank 0)
        hi = np.searchsorted(starts_global, st_e, side="right")
        lo = np.searchsorted(starts_global, (pos // 128) * 128, side="right")
        ord_e = np.maximum(hi - lo, 0)
        assert ord_e.max(initial=0) <= 3
        qidx_slots = np.zeros(S, dtype=np.int16)
        ids_slots = np.full(S, PAD_ID, dtype=np.float32)
        qidx_slots[pos] = qi_s[a:b]
        ids_slots[pos] = dloc_s[a:b] + 128.0 * ord_e
        # idx image: slot s consumed at (partition s%16, col s//16), x8 replicas
        gidx_imgs.append(np.ascontiguousarray(
            np.tile(qidx_slots.reshape(-1, 16).T, (8, 1))))
        ids_imgs.append(np.ascontiguousarray(
            ids_slots.reshape(M, 128).T))

    cnt_per_core = np.bincount(dst, minlength=cfg["N"]).reshape(NCORES, NSH)
    return dict(alloc=alloc, run_len=run_len, run_alloc=run_alloc,
                run_base=run_base, M=M, gidx=gidx_imgs, ids=ids_imgs,
                cnt=cnt_per_core.astype(np.float32))


def _balance_nodes(cfg, inputs):
    """Per dst type j: assign nodes to (core, tile, slot) so that each
    (tile, src-type, quarter) in-degree is balanced across the 8 cores
    (greedy L2 potential within each tile octet). Shrinks the max-over-
    cores slot allocation toward the mean, cutting gather descriptors.

    Returns posmap[j]: node -> global position = core*NSH + tile*128 + s.
    """
    C, N, NCORES, NSH, NT, NQ, QS = (
        cfg["C"], cfg["N"], cfg["NCORES"], cfg["NSH"], cfg["NT"],
        cfg["NQ"], cfg["QS"])
    D = C * NQ
    posmaps = []
    for j in range(C):
        degv = np.zeros(N * D, dtype=np.int64)
        for i in range(C):
            e = np.asarray(inputs[f"e{i}{j}"])
            src = e[0].astype(np.int64)
            dst = e[1].astype(np.int64)
            degv += np.bincount(dst * D + i * NQ + src // QS,
                                minlength=N * D)
        degv = degv.reshape(N, D).astype(np.float64)
        order = np.argsort(-degv.sum(1), kind="stable")
        posmap = np.empty(N, dtype=np.int64)
        # stride degree-sorted nodes across the full tiles so every tile
        # gets a similar degree mix (keeps per-tile totals, and thus the
        # per-group SBUF gather buffers, uniform); the partial last tile
        # takes the lowest-degree tail
        full = NT - 1
        for T in range(NT):
            cap = min(128, NSH - 128 * T)
            if T < full:
                nodes = order[T + full * np.arange(NCORES * cap)]
            else:
                nodes = order[full * NCORES * 128:]
                assert len(nodes) == NCORES * cap
            loads = np.zeros((NCORES, D))
            fill = np.zeros(NCORES, dtype=np.int64)
            for n in nodes:
                x = degv[n]
                scores = loads @ x
                scores[fill >= cap] = np.inf
                c = int(np.argmin(scores))
                posmap[n] = c * NSH + T * 128 + fill[c]
                loads[c] += x
                fill[c] += 1
        posmaps.append(posmap)
    return posmaps


def host_prep(cfg, inputs):
    C, NCORES, NT, NSH, NTP = (
        cfg["C"], cfg["NCORES"], cfg["NT"], cfg["NSH"], cfg["NTP"])
    posmaps = _balance_nodes(cfg, inputs)
    invs = []
    for j in range(C):
        inv = np.empty(cfg["N"], dtype=np.int64)
        inv[posmaps[j]] = np.arange(cfg["N"])
        invs.append(inv)
    pairs = {}
    for i in range(C):
        for j in range(C):
            e = np.asarray(inputs[f"e{i}{j}"])
            er = np.stack([e[0].astype(np.int64),
                           posmaps[j][e[1].astype(np.int64)]])
            pairs[(i, j)] = _prep_pair(cfg, er)

    shared = {}
    for i in range(C):
        x = np.asarray(inputs[f"x{i}"], dtype=np.float32)
        xp = np.zeros((cfg["N"], 128), dtype=NP_FP16)
        xp[:, :64] = x.astype(NP_FP16)
        shared[f"xp{i}"] = xp
    shared["iota4"] = np.tile(
        np.arange(512, dtype=np.float32).astype(NP_FP16), (128, 1))
    shared["ones1x64"] = np.ones((1, 64), dtype=NP_BF16)
    Wl = np.asarray(inputs["Wl"], np.float32)
    Wr = np.asarray(inputs["Wr"], np.float32)
    bl = np.asarray(inputs["bl"], np.float32)
    linW = np.asarray(inputs["linW"], np.float32)
    linb = np.asarray(inputs["linb"], np.float32)
    for i in range(C):
        for j in range(C):
            shared[f"wl_{i}{j}"] = np.ascontiguousarray(Wl[i, j])
            shared[f"wr_{i}{j}"] = np.ascontiguousarray(Wr[i, j])
    for j in range(C):
        shared[f"linwT_{j}"] = np.ascontiguousarray(linW[j].T)
        shared[f"blc_{j}"] = np.ascontiguousarray(
            np.concatenate([bl[i, j] for i in range(C)]).reshape(-1, 1))
        shared[f"linb_{j}"] = np.ascontiguousarray(linb[j].reshape(1, -1))

    in_maps = []
    for cidx in range(NCORES):
        m = dict(shared)
        for i in range(C):
            for j in range(C):
                m[f"gidx_{i}{j}"] = pairs[(i, j)]["gidx"][cidx]
                m[f"ids_{i}{j}"] = pairs[(i, j)]["ids"][cidx]
        for j in range(C):
            # recip image, tile-major flat: f = 384*t + 128*i + d
            rec = np.ones((NT, C, 128), dtype=np.float32)
            for i in range(C):
                cp = np.zeros(NTP, dtype=np.float32)
                cp[:NSH] = pairs[(i, j)]["cnt"][cidx]
                rec[:, i, :] = 1.0 / np.maximum(cp, 1.0).reshape(NT, 128)
            m[f"recflat_{j}"] = rec.reshape(1, -1).astype(NP_BF16)
            xTb = np.zeros((65, NTP), dtype=NP_BF16)
            x = np.asarray(inputs[f"x{j}"], np.float32)
            rows = invs[j][cidx * NSH:(cidx + 1) * NSH]
            xTb[:64, :NSH] = x[rows].T.astype(NP_BF16)
            xTb[64, :] = 1.0
            m[f"xTb_{j}"] = xTb
        in_maps.append(m)

    struct = {"posmap": posmaps}
    for i in range(C):
        for j in range(C):
            p = pairs[(i, j)]
            struct[f"s_{i}{j}"] = {k: p[k] for k in
                                   ("alloc", "run_len", "run_alloc",
                                    "run_base", "M")}
    return in_maps, struct


# ---------------------------------------------------------------- bass build
def build_bass(cfg, struct):
    C, NT, NTP, NQ, QS, G, NG, DEGC = (
        cfg["C"], cfg["NT"], cfg["NTP"], cfg["NQ"], cfg["QS"], cfg["G"],
        cfg["NG"], cfg["DEGC"])
    N = cfg["N"]
    nc = bacc_mod.Bacc("TRN2", target_bir_lowering=False)

    xp_p = [nc.declare_dram_parameter(f"xp{i}", [N, 128], FP16, isOutput=False)
            for i in range(C)]
    iota_p = nc.declare_dram_parameter("iota4", [128, 512], FP16, isOutput=False)
    ones_p = nc.declare_dram_parameter("ones1x64", [1, 64], BF16, isOutput=False)
    gidx_p, ids_p, wl_p, wr_p = {}, {}, {}, {}
    for i in range(C):
        for j in range(C):
            M = struct[f"s_{i}{j}"]["M"]
            gidx_p[(i, j)] = nc.declare_dram_parameter(
                f"gidx_{i}{j}", [128, 8 * M], I16, isOutput=False)
            ids_p[(i, j)] = nc.declare_dram_parameter(
                f"ids_{i}{j}", [128, M], F32, isOutput=False)
            wl_p[(i, j)] = nc.declare_dram_parameter(
                f"wl_{i}{j}", [64, 64], F32, isOutput=False)
            wr_p[(i, j)] = nc.declare_dram_parameter(
                f"wr_{i}{j}", [64, 64], F32, isOutput=False)
    linwT_p, blc_p, linb_p, rec_p, xTb_p, out_p = {}, {}, {}, {}, {}, {}
    for j in range(C):
        linwT_p[j] = nc.declare_dram_parameter(
            f"linwT_{j}", [192, 64], F32, isOutput=False)
        blc_p[j] = nc.declare_dram_parameter(f"blc_{j}", [192, 1], F32, isOutput=False)
        linb_p[j] = nc.declare_dram_parameter(f"linb_{j}", [1, 64], F32, isOutput=False)
        rec_p[j] = nc.declare_dram_parameter(
            f"recflat_{j}", [1, 128 * DEGC], BF16, isOutput=False)
        xTb_p[j] = nc.declare_dram_parameter(f"xTb_{j}", [65, NTP], BF16, isOutput=False)
        out_p[j] = nc.declare_dram_parameter(f"out_{j}", [64, NTP], F32, isOutput=True)

    # ---- static slot structure bookkeeping (per pair) -------------------
    st = {(i, j): struct[f"s_{i}{j}"] for i in range(C) for j in range(C)}
    S_g = {k: s["run_alloc"].sum(axis=1) // 128 for k, s in st.items()}  # [NG]
    goff = {k: s["run_base"][:, 0] // 128 for k, s in st.items()}        # [NG]
    SGMAX = int(max(int(S_g[k].max()) for k in st))

    def tile_jobs(k, g, t):
        """Matmul jobs for (pair k, group g, tile t): list of
        (group-local chunk, iota-bank ordinal). All matmuls contract the
        full 128 partitions; the one-hot compare vs iota bank `ord`
        zeroes slots belonging to other tiles sharing the chunk."""
        s = st[k]
        rb0 = s["run_base"][g]
        alloc = s["alloc"][g]                 # [NQ, G]
        jobs = []
        for q in range(NQ):
            base = int(rb0[q] - rb0[0])
            starts = base + np.concatenate(
                [[0], np.cumsum(alloc[q])])[:-1]
            s0 = int(starts[t])
            s1 = s0 + int(alloc[q, t])
            for c in range(s0 // 128, (s1 - 1) // 128 + 1):
                cb = 128 * c
                ordn = int(((starts > cb) & (starts <= s0)).sum())
                assert ordn <= 3
                jobs.append((c, ordn))
        return jobs

    from contextlib import ExitStack
    with TileContext(nc) as tc, ExitStack() as es:
        def pool(name, bufs, space="SBUF"):
            return es.enter_context(
                tc.tile_pool(name=name, bufs=bufs, space=space))
        cpool = pool("consts", 1)
        wpool = pool("wsb", 1)
        wlpool = pool("wload", 2)
        xtpool = pool("xt", 2)
        gixpool = pool("gidx", 2)
        idspool = pool("ids", 2)
        xgpool = pool("xg", 2)
        ohpool = pool("oh", 8)
        meanpool = pool("mean", 2)
        rbpool = pool("rb", 2)
        recpool = pool("rc", 2)
        outpool = pool("osb", 2)
        psa = pool("ps_a", 2, "PSUM")
        psb = pool("ps_b", 2, "PSUM")
        psc = pool("ps_c", 2, "PSUM")
        psrb = pool("ps_rb", 1, "PSUM")
        pss2 = pool("ps_s2", 1, "PSUM")
        pseg = [psa, psb, psc]

        # ---- constants
        iota_sb = cpool.tile([128, 512], FP16, name="iota_sb")
        nc.sync.dma_start(out=iota_sb[:, :], in_=iota_p[:, :])
        ones_sb = cpool.tile([1, 64], BF16, name="ones_sb")
        nc.sync.dma_start(out=ones_sb[:, :], in_=ones_p[:, :])

        # ---- fold weights: A_ij, Cp_j (device-side, tiny fp32 matmuls).
        # Emitted after the first group's gathers so the fold's PE work
        # overlaps the pipeline fill instead of delaying it.
        A_sb, Cp_sb = {}, {}

        def emit_fold():
          for j in range(C):
            lw_t = []
            for i in range(C):
                lwi = wlpool.tile([64, 64], F32, tag=f"lw{i}")
                nc.sync.dma_start(out=lwi[:, :],
                                  in_=linwT_p[j][64 * i:64 * (i + 1), :])
                lw_t.append(lwi)

            Cp = wpool.tile([65, 64], BF16, tag=f"cp{j}", name=f"cp{j}")
            Cp_sb[j] = Cp
            ps = pss2.tile([64, 64], F32, tag="ps2")
            for i in range(C):
                w = wlpool.tile([64, 64], F32, tag="w")
                nc.sync.dma_start(out=w[:, :], in_=wr_p[(i, j)][:, :])
                nc.tensor.matmul(ps[:, :], w[:, :], lw_t[i][:, :],
                                 start=(i == 0), stop=(i == C - 1))
            nc.scalar.copy(out=Cp[0:64, :], in_=ps[:, :])
            psb_row = pss2.tile([1, 64], F32, tag="ps2")
            blc_t = []
            for i in range(C):
                bci = wlpool.tile([64, 1], F32, tag=f"blc{i}")
                nc.sync.dma_start(out=bci[:, :],
                                  in_=blc_p[j][64 * i:64 * (i + 1), :])
                blc_t.append(bci)
            lb = wlpool.tile([1, 64], F32, tag="lb")
            nc.sync.dma_start(out=lb[:, :], in_=linb_p[j][:, :])
            one1 = wlpool.tile([1, 1], F32, tag="one1")
            nc.vector.memset(one1[:, :], 1.0)
            for i in range(C):
                nc.tensor.matmul(psb_row[:, :], blc_t[i][:, :], lw_t[i][:, :],
                                 start=(i == 0), stop=False)
            nc.tensor.matmul(psb_row[:, :], one1[:, :], lb[:, :],
                             start=False, stop=True)
            nc.scalar.copy(out=Cp[64:65, :], in_=psb_row[:, :])
            for i in range(C):
                ps2 = pss2.tile([64, 64], F32, tag="ps2")
                w = wlpool.tile([64, 64], F32, tag="w")
                nc.sync.dma_start(out=w[:, :], in_=wl_p[(i, j)][:, :])
                nc.tensor.matmul(ps2[:, :], w[:, :], lw_t[i][:, :],
                                 start=True, stop=True)
                A = wpool.tile([64, 64], BF16, tag=f"a{i}{j}", name=f"a{i}{j}")
                A_sb[(i, j)] = A
                nc.scalar.copy(out=A[:, :], in_=ps2[:, :])

        # ---- main loop: j-major, then tile groups
        xg_inits = [0, 0, 0]
        for j in range(C):
            xt16 = xtpool.tile([65, NTP], BF16, tag="xt")
            nc.sync.dma_start(out=xt16[:, :], in_=xTb_p[j][:, :])

            for g in range(NG):
                # For the very last group, split each quarter gather in
                # two chunk halves: tiles 0..3 compute against the first
                # half while the second half is still in flight, which
                # shrinks the end-of-kernel DMA->compute tail.
                split_last = (j == C - 1 and g == NG - 1)
                deferred = []
                xg_i, ids_i = [], []
                for i in range(C):
                    k = (i, j)
                    S = int(S_g[k][g])
                    go = int(goff[k][g])
                    rb0 = st[k]["run_base"][g]
                    ra = st[k]["run_alloc"][g]
                    rl = st[k]["run_len"][g]
                    gix = gixpool.tile([128, 8 * SGMAX], I16, tag=f"gi{i}")
                    nc.sync.dma_start(
                        out=gix[:, 0:8 * S],
                        in_=gidx_p[k][:, 8 * go:8 * (go + S)])
                    idst = idspool.tile([128, SGMAX], F32, tag=f"id{i}")
                    nc.sync.dma_start(
                        out=idst[:, 0:S], in_=ids_p[k][:, go:go + S])
                    xg = xgpool.tile([128, 128 * SGMAX], BF16, tag=f"xg{i}")
                    for q in range(NQ):
                        cq = int(ra[q]) // 128
                        ni = int(rl[q])
                        if cq == 0:
                            continue
                        c0 = int(rb0[q] - rb0[0]) // 128
                        def emit(xg=xg, gix=gix, i=i, q=q, c0=c0,
                                 cq=cq, ni=ni):
                            if ni < 128 * cq:
                                # zero the ragged last chunk so slots past
                                # num_idxs never expose NaN bits to matmuls
                                nc.gpsimd.memset(
                                    xg[:, 128 * (c0 + cq - 1):
                                       128 * (c0 + cq)], 0.0)
                            nc.gpsimd.dma_gather(
                                out_ap=xg[:, 128 * c0:128 * (c0 + cq)]
                                .rearrange("p (m e) -> p m e", e=128),
                                in_ap=xp_p[i][QS * q:QS * (q + 1), :],
                                idxs_ap=gix[:, 8 * c0:
                                            8 * c0 + (ni + 15) // 16],
                                num_idxs=ni,
                                num_idxs_reg=ni,
                                elem_size=128,
                                single_packet=False,
                            )
                        if split_last and cq >= 2:
                            cq1 = max(1, (cq * 4) // 7)
                            emit(cq=cq1, ni=128 * cq1)
                            deferred.append(
                                lambda e=emit, c0=c0 + cq1, cq=cq - cq1,
                                ni=ni - 128 * cq1:
                                e(c0=c0, cq=cq, ni=ni))
                        else:
                            emit()
                    xg_i.append(xg)
                    ids_i.append(idst)

                for fn in deferred:
                    fn()

                if j == 0 and g == 0:
                    emit_fold()

                rc3 = recpool.tile([1, 384 * G], BF16, tag="rc")
                nc.sync.dma_start(
                    out=rc3[:, :],
                    in_=rec_p[j][0:1, 384 * G * g:384 * G * (g + 1)])
                osb = outpool.tile([64, 128 * G], F32, tag="osb")

                for t in range(G):
                    T = g * G + t
                    ps_i = []
                    for i in range(C):
                        jobs = tile_jobs((i, j), g, t)
                        assert jobs, f"empty segment pair=({i},{j}) tile={T}"
                        nmm = len(jobs)
                        ps = pseg[i].tile([64, 128], F32, tag=f"seg{i}")
                        ps_i.append(ps)
                        for mm, (ck, ordn) in enumerate(jobs):
                            oh = ohpool.tile([128, 128], FP16, tag="oh")
                            nc.vector.tensor_scalar(
                                oh[:, :],
                                iota_sb[:, 128 * ordn:128 * (ordn + 1)],
                                ids_i[i][:, ck:ck + 1], None,
                                mybir.AluOpType.is_equal)
                            nc.tensor.matmul(
                                ps[:, :],
                                xg_i[i][:, 128 * ck:128 * ck + 64],
                                oh[:, :],
                                start=(mm == 0), stop=(mm == nmm - 1))

                    rb_ps = psrb.tile([64, 384], F32, tag="rbps")
                    nc.tensor.matmul(rb_ps[:, :], ones_sb[:, :],
                                     rc3[:, 384 * t:384 * (t + 1)],
                                     start=True, stop=True)
                    rb = rbpool.tile([64, 384], F32, tag="rb")
                    nc.scalar.copy(out=rb[:, :], in_=rb_ps[:, :])

                    ps2 = pss2.tile([64, 128], F32, tag="ps2")
                    nc.tensor.matmul(ps2[:, :], Cp_sb[j][:, :],
                                     xt16[:, 128 * T:128 * (T + 1)],
                                     start=True, stop=False)
                    for i in range(C):
                        mean = meanpool.tile([64, 128], BF16, tag="mean")
                        nc.vector.tensor_tensor(
                            mean[:, :], ps_i[i][:, :],
                            rb[:, 128 * i:128 * (i + 1)],
                            mybir.AluOpType.mult)
                        nc.tensor.matmul(ps2[:, :], A_sb[(i, j)][:, :],
                                         mean[:, :],
                                         start=False, stop=(i == C - 1))
                    nc.scalar.copy(out=osb[:, 128 * t:128 * (t + 1)],
                                   in_=ps2[:, :])
                nc.sync.dma_start(
                    out=out_p[j][:, 128 * G * g:128 * G * (g + 1)],
                    in_=osb[:, :])
    nc.compile()
    return nc


# ---------------------------------------------------------------- entry point
def unshard(cfg, struct, results):
    C, NCORES, NSH = cfg["C"], cfg["NCORES"], cfg["NSH"]
    out = np.empty((C, cfg["N"], 64), dtype=np.float32)
    for j in range(C):
        pos = np.empty((cfg["N"], 64), dtype=np.float32)
        for cidx in range(NCORES):
            pos[cidx * NSH:(cidx + 1) * NSH, :] = \
                results[cidx][f"out_{j}"][:, :NSH].T
        out[j] = pos[struct["posmap"][j]]
    return out


def run(cfg, inputs, trace=False):
    cfg = _derive(cfg)
    in_maps, struct = host_prep(cfg, inputs)
    nc = build_bass(cfg, struct)
    res = run_bass_kernel_spmd(nc, in_maps, list(range(cfg["NCORES"])),
                               trace=trace)
    out = unshard(cfg, struct, res.results)
    return out, res


def kernel(**inputs):
    out, _ = run(default_cfg(), inputs)
    return out
